# revision 20
# baseline (speedup 1.0000x reference)
"""Trainium2 Bass kernel for nn_AdapterController (moe_routing).

Math (per sentence):
  z = LayerNorm(x) * g + b                      [S, D]
  probs = softmax(BN(mean_s z) @ Wr + br)       [E]
  idx = argmax(probs); gate = probs[idx]
  y = (relu(z @ W_down[idx] + b_down[idx]) @ W_up[idx] + b_up[idx]) * gate

Strategy: data-parallel over batch (8 sentences per core, no collectives).
DMA carries x-in (f32->bf16 SWDGE cast) + y-out; on-chip:
  - LN stats via bn_stats on bf16 x (DVE).
  - norm is FUSED into the PE transpose: zT' = xb_tile^T @ diag(rs_t)
    (regular matmuls into f32 PSUM). The mean-shift term (-mu*rs = nmr)
    is folded into mm1 as a K=2 rank-2 matmul with a host-precomputed
    [b_down ; colsum(W_down)] table, and into the router logits with a
    host colsum(Wr) constant.
  - zT PSUM->SBUF copies on ACT with accum_out => router token-sums free.
  - expert selection via GPSIMD ap_gather with an on-device argmax index
    (runtime-register DMA offsets crash under this runtime; ap_gather
    reads the index from SBUF instead). Gate is applied in the y copies.
  - mm1 -> hT psum, relu-cast on ACT; mm2 -> y psum, gate-cast copies on
    ACT/DVE; bf16->f32 cast in the output DMA (SWDGE on Pool).
  - emission is software-pipelined (front(b); back(b-1)).
"""

import sys

if "/opt/trn_rl_repo" not in sys.path:
    sys.path.insert(0, "/opt/trn_rl_repo")

from contextlib import ExitStack

import ml_dtypes
import numpy as np

import concourse.bacc as bacc
import concourse.bass as bass
import concourse.tile as tile
from concourse import mybir
from concourse.bass_utils import run_bass_kernel_spmd

B, S, D, H, E = 64, 1024, 1024, 64, 8
NCORES = 8
BLOC = B // NCORES
P = 128
TC = S // P  # token chunks per sentence
DC = D // P  # d chunks
EPS = 1e-5
FP32 = mybir.dt.float32
BF16 = mybir.dt.bfloat16
U16 = mybir.dt.uint16
I16 = mybir.dt.int16

N_Y_DVE = 6  # of 16 y copies on DVE (rest ACT)

_CACHE = {}


def _build_kernel():
    nc = bacc.Bacc(
        "TRN2",
        target_bir_lowering=False,
        debug=False,
        enable_asserts=False,
        num_devices=NCORES,
    )
    x_ext = nc.dram_tensor("x", [BLOC, S, D], FP32, kind="ExternalInput").ap()
    wr_ext = nc.dram_tensor("wr", [DC, P, E], FP32, kind="ExternalInput").ap()
    wrc_ext = nc.dram_tensor("wrc", [P, E], FP32, kind="ExternalInput").ap()
    c_ext = nc.dram_tensor("c", [1, E], FP32, kind="ExternalInput").ap()
    wd_ext = nc.dram_tensor("wd", [E, P, DC * H], BF16, kind="ExternalInput").ap()
    wu_ext = nc.dram_tensor("wu", [E, 80, D], BF16, kind="ExternalInput").ap()
    bw_ext = nc.dram_tensor("bw", [16, E, H], BF16, kind="ExternalInput").ap()
    ident_ext = nc.dram_tensor("ident", [P, P], BF16, kind="ExternalInput").ap()
    iota_ext = nc.dram_tensor("iota16", [P, 1], U16, kind="ExternalInput").ap()
    out_ext = nc.dram_tensor("out", [BLOC, S, D], FP32, kind="ExternalOutput").ap()

    with tile.TileContext(nc) as tc:
        _body(tc, out_ext, x_ext, wr_ext, wrc_ext, c_ext, wd_ext, wu_ext,
              bw_ext, ident_ext, iota_ext)

    nc.compile()
    return nc


def _body(tc, out_ext, x_ext, wr_ext, wrc_ext, c_ext, wd_ext, wu_ext,
          bw_ext, ident_ext, iota_ext):
    nc = tc.nc
    with ExitStack() as ctx:
        consts = ctx.enter_context(tc.tile_pool(name="consts", bufs=1))
        x_pool = ctx.enter_context(tc.tile_pool(name="xp", bufs=4))
        zt_pool = ctx.enter_context(tc.tile_pool(name="ztp", bufs=2))
        y_pool = ctx.enter_context(tc.tile_pool(name="yp", bufs=3))
        w_pool = ctx.enter_context(tc.tile_pool(name="wp", bufs=2))
        st_pool = ctx.enter_context(tc.tile_pool(name="stp", bufs=3))
        pp_zt = ctx.enter_context(tc.tile_pool(name="ppzt", bufs=3, space="PSUM"))
        pp_y = ctx.enter_context(tc.tile_pool(name="ppy", bufs=4, space="PSUM"))
        pp_r = ctx.enter_context(tc.tile_pool(name="ppr", bufs=1, space="PSUM"))

        # constants
        ones_row = consts.tile([1, P], FP32)
        nc.vector.memset(ones_row, 1.0)
        one_f32 = consts.tile([1, 1], FP32)
        nc.vector.memset(one_f32, 1.0)
        c_sb = consts.tile([1, E], FP32)
        nc.sync.dma_start(c_sb, c_ext)
        wr_sb = consts.tile([P, DC, E], FP32)
        nc.sync.dma_start(wr_sb, wr_ext.rearrange("dc p e -> p dc e"))
        wrc_sb = consts.tile([P, E], FP32)
        nc.sync.dma_start(wrc_sb, wrc_ext)
        ident_sb = consts.tile([P, P], BF16)
        nc.sync.dma_start(ident_sb, ident_ext)
        iota16 = consts.tile([P, 1], U16)
        nc.sync.dma_start(iota16, iota_ext)
        bw_all = consts.tile([16, E, H], BF16)
        nc.sync.dma_start(bw_all, bw_ext)
        wd_all = consts.tile([P, E, DC * H], BF16)
        nc.sync.dma_start(wd_all, wd_ext.rearrange("e p x -> p e x"))
        wu_all = consts.tile([80, E, D], BF16)
        nc.sync.dma_start(wu_all, wu_ext.rearrange("e h d -> h e d"))
        # persistent parity tiles: hT (ones row) and [ones ; nmr] K=2 rhs
        hT_tiles, onmr_tiles = [], []
        for i in range(2):
            hT_p = consts.tile([H + 1, S], BF16, tag=f"hT{i}")
            nc.vector.memset(hT_p[H : H + 1], 1.0)
            hT_tiles.append(hT_p)
            onmr_p = consts.tile([2, S], BF16, tag=f"onmr{i}")
            nc.vector.memset(onmr_p[0:1], 1.0)
            onmr_tiles.append(onmr_p)

        state0 = {}
        state = {}
        state2 = {}
        xtiles = {}

        def load(b):
            # ---- load x as bf16 via SWDGE cast (Pool) ----
            if b >= BLOC:
                return
            x_src = x_ext[b].rearrange("(tc p) d -> p tc d", p=P)
            xb = x_pool.tile([P, TC, D], BF16, tag="xb")
            for i in range(4):
                nc.gpsimd.dma_start(
                    xb[:, 2 * i : 2 * i + 2], x_src[:, 2 * i : 2 * i + 2]
                )
            xtiles[b] = xb

        def front_stats(b):
            xb = xtiles[b]

            # ---- layernorm stats (DVE, bf16 input) ----
            mv = st_pool.tile([P, TC, 2], FP32)
            for t in range(TC):
                stats = st_pool.tile([P, 2, 6], FP32, tag="bnstats")
                xv = xb[:, t].rearrange("p (g f) -> p g f", g=2)
                nc.vector.bn_stats(stats[:, 0], xv[:, 0])
                nc.vector.bn_stats(stats[:, 1], xv[:, 1])
                nc.vector.bn_aggr(mv[:, t], stats)
            # rs = rsqrt(var + eps): linear seed + 3 Newton iterations (DVE)
            rs = st_pool.tile([P, TC], FP32)
            vv = st_pool.tile([P, TC], FP32)
            nc.vector.tensor_scalar_add(vv, mv[:, :, 1], float(EPS))
            nc.vector.tensor_scalar(
                rs, vv, -0.5, 1.5, mybir.AluOpType.mult, mybir.AluOpType.add
            )
            tnw = st_pool.tile([P, TC], FP32)
            for _ in range(2):
                nc.vector.tensor_mul(tnw, rs, rs)
                nc.vector.tensor_mul(tnw, tnw, vv)
                nc.vector.tensor_scalar(
                    tnw, tnw, -0.5, 1.5, mybir.AluOpType.mult, mybir.AluOpType.add
                )
                nc.vector.tensor_mul(rs, rs, tnw)
            nmr = st_pool.tile([P, TC], FP32)
            nc.vector.tensor_mul(nmr, mv[:, :, 0], rs)
            nc.vector.tensor_scalar_mul(nmr, nmr, -1.0)

            # diag(rs_t) tiles (DVE) for the fused norm-transpose
            diag = st_pool.tile([P, TC, P], BF16, tag="diag")
            for t in range(TC):
                nc.vector.tensor_scalar_mul(diag[:, t], ident_sb, rs[:, t : t + 1])

            # nmr summaries: per-partition partial sums (for router corr)
            nmr_part = st_pool.tile([P, 1], FP32)
            nc.vector.reduce_sum(nmr_part, nmr, axis=mybir.AxisListType.X)
            state0[b] = (rs, nmr, nmr_part, diag)

        def ft_back(b, bp):
            # interleaves front(b)'s fused norm-transposes with back(bp)'s
            # mm1/mm2 so the PE queue is one long ready stream
            chunks = back_chunks(bp)

            def bc(i):
                if i < len(chunks):
                    chunks[i]()

            if b is None:
                for i in range(len(chunks)):
                    bc(i)
                return
            xb = xtiles.pop(b)
            rs, nmr, nmr_part, diag = state0.pop(b)
            # ---- fused norm+transpose (PE) + ACT copies w/ accum ----
            zT_sb = zt_pool.tile([P, DC, S], BF16)
            sacc2 = st_pool.tile([P, 2, DC], FP32)
            for dc in range(DC):
                for g in range(2):
                    pzt = pp_zt.tile([P, 512], FP32, tag="zt")
                    for tt in range(4):
                        t = 4 * g + tt
                        nc.tensor.matmul(
                            pzt[:, tt * P : (tt + 1) * P],
                            xb[:, t, dc * P : (dc + 1) * P],
                            diag[:, t],
                            start=True,
                            stop=True,
                        )
                    nc.scalar.activation(
                        zT_sb[:, dc, g * 512 : (g + 1) * 512],
                        pzt,
                        mybir.ActivationFunctionType.Copy,
                        accum_out=sacc2[:, g, dc : dc + 1],
                    )
                if dc % 2 == 1:
                    bc(dc // 2)
            sacc = st_pool.tile([P, DC], FP32)
            nc.vector.tensor_add(sacc, sacc2[:, 0], sacc2[:, 1])

            # ---- logits = s @ Wr_f + c + nmrsum*colsum(Wr_f)  (PE) ----
            ps_l = pp_r.tile([1, E], FP32, tag="ps_small")
            for dc in range(DC):
                nc.tensor.matmul(
                    ps_l, sacc[:, dc : dc + 1], wr_sb[:, dc], start=(dc == 0),
                    stop=False,
                )
            nc.tensor.matmul(ps_l, nmr_part, wrc_sb, start=False, stop=False)
            nc.tensor.matmul(ps_l, one_f32, c_sb, start=False, stop=True)
            logits_sb = st_pool.tile([1, E], FP32)
            nc.scalar.copy(logits_sb, ps_l)
            for i in range(4, len(chunks)):
                bc(i)

            state[b] = (zT_sb, logits_sb, nmr)

        def front_post(b):
            zT_sb, logits_sb, nmr = state.pop(b)
            # nmr as a [1, S] bf16 row (token-major flatten) for the mm1 K=2 bias
            nmr_bf = st_pool.tile([P, TC], BF16, tag="nmrbf")
            nc.vector.tensor_copy(nmr_bf, nmr)
            ps_nt = pp_r.tile([TC, P], BF16, tag="ps_small")
            nc.tensor.transpose(ps_nt, nmr_bf, ident_sb)
            nmrT = st_pool.tile([TC, P], BF16, tag="nmrT")
            nc.scalar.copy(nmrT, ps_nt)
            onmr = onmr_tiles[b % 2]
            nc.sync.dma_start(
                onmr[1:2].rearrange("a (tc p) -> a tc p", p=P), nmrT
            )
            # ---- gating + expert index (DVE) ----
            mx8 = st_pool.tile([1, E], FP32)
            ix8 = st_pool.tile([1, E], U16)
            nc.vector.max_with_indices(mx8, ix8, logits_sb)
            u = st_pool.tile([1, E], FP32)
            nc.vector.tensor_scalar(
                u, logits_sb, mx8[0:1, 0:1], None, mybir.AluOpType.subtract
            )
            ex = st_pool.tile([1, E], FP32)
            nc.vector.tensor_scalar(
                ex, u, 0.25, 1.0, mybir.AluOpType.mult, mybir.AluOpType.add
            )
            for coef in (3.0, 2.0, 1.0):
                nc.vector.tensor_mul(ex, ex, u)
                nc.vector.tensor_scalar(
                    ex, ex, 1.0 / coef, 1.0, mybir.AluOpType.mult, mybir.AluOpType.add
                )
            denom = st_pool.tile([1, 1], FP32)
            nc.vector.tensor_reduce(
                denom, ex, axis=mybir.AxisListType.X, op=mybir.AluOpType.add
            )
            gate = st_pool.tile([1, 1], FP32)
            nc.vector.reciprocal(gate, denom)
            ix4 = st_pool.tile([1, 1], U16)
            nc.vector.tensor_scalar_mul(ix4, ix8[0:1, 0:1], 4)

            # index row: 4*idx + (p%16 capped at 3), broadcast to 128 parts
            ixb = st_pool.tile([P, 1], U16)
            nc.gpsimd.partition_broadcast(ixb, ix4)
            nc.vector.tensor_add(ixb, ixb, iota16)

            # ---- expert selection via ap_gather (Pool) ----
            wd_eff = w_pool.tile([P, 4, P], BF16, tag="wd")
            nc.gpsimd.ap_gather(
                wd_eff,
                wd_all.rearrange("p e (q f) -> p (e q) f", q=4),
                ixb.bitcast(I16),
                channels=P, num_elems=4 * E, d=P, num_idxs=4,
            )
            wu_eff = w_pool.tile([80, 4, 256], BF16, tag="wu")
            nc.gpsimd.ap_gather(
                wu_eff,
                wu_all.rearrange("h e (q f) -> h (e q) f", q=4),
                ixb[0:80].bitcast(I16),
                channels=80, num_elems=4 * E, d=256, num_idxs=4,
            )
            bw_sel = w_pool.tile([16, 4, 16], BF16, tag="bw")
            nc.gpsimd.ap_gather(
                bw_sel,
                bw_all.rearrange("c e (q f) -> c (e q) f", q=4),
                ixb[0:16].bitcast(I16),
                channels=16, num_elems=4 * E, d=16, num_idxs=4,
            )

            # gate broadcast to 128 partitions (Pool, tiny)
            gate_bc = st_pool.tile([P, 1], FP32)
            nc.gpsimd.partition_broadcast(gate_bc, gate)

            state2[b] = (zT_sb, wd_eff, wu_eff, bw_sel, gate_bc)

        def back_chunks(b):
            if b is None:
                return []
            zT_sb, wd_eff, wu_eff, bw_sel, gate_bc = state2.pop(b)
            hT = hT_tiles[b % 2]
            onmr = onmr_tiles[b % 2]
            wd_flat = wd_eff.rearrange("p q f -> p (q f)")
            wu_flat = wu_eff.rearrange("p q f -> p (q f)")
            bw_flat = bw_sel.rearrange("p q f -> p (q f)")
            y_dst = out_ext[b].rearrange("(tc p) d -> p tc d", p=P)
            y_hs = []
            for i in range(2):
                y_half = y_pool.tile([P, TC // 2, D], BF16, tag="yh")
                y_hs.append(y_half)

            def mm1(half):
                hsl = slice(half * 512, (half + 1) * 512)
                ps_hT = pp_y.tile([H, 512], FP32, tag="ps")
                for dc in range(DC):
                    nc.tensor.matmul(
                        ps_hT,
                        wd_flat[:, dc * H : (dc + 1) * H],
                        zT_sb[:, dc, hsl],
                        start=(dc == 0),
                        stop=False,
                    )
                nc.tensor.matmul(
                    ps_hT, bw_flat[0:2], onmr[:, hsl], start=False, stop=True
                )
                nc.scalar.activation(
                    hT[0:H, hsl], ps_hT, mybir.ActivationFunctionType.Relu
                )

            def mm2(trange, store_half=None):
                for t in trange:
                    y_sb = y_hs[t // (TC // 2)]
                    yt = t % (TC // 2)
                    for half in range(2):
                        hs = slice(half * 512, (half + 1) * 512)
                        ps_y = pp_y.tile([P, 512], FP32, tag="ps")
                        nc.tensor.matmul(
                            ps_y,
                            hT[:, t * P : (t + 1) * P],
                            wu_flat[0 : H + 1, hs],
                            start=True,
                            stop=True,
                        )
                        if (2 * t + half) % 16 < N_Y_DVE:
                            nc.vector.tensor_scalar_mul(
                                y_sb[:, yt, hs], ps_y, gate_bc
                            )
                        else:
                            nc.scalar.mul(y_sb[:, yt, hs], ps_y, gate_bc)
                if store_half is not None:
                    h0 = store_half * (TC // 2)
                    nc.gpsimd.dma_start(
                        y_dst[:, h0 : h0 + TC // 2], y_hs[store_half]
                    )

            return [
                lambda: mm1(0),
                lambda: mm1(1),
                lambda: mm2(range(0, 4), store_half=0),
                lambda: mm2(range(4, 8), store_half=1),
            ]

        load(0)
        load(1)
        front_stats(0)
        load(2)
        for k in range(BLOC):
            if k >= 1:
                front_post(k - 1)
            if k + 1 < BLOC:
                front_stats(k + 1)
            ft_back(k, k - 1 if k >= 1 else None)
            load(k + 3)
        front_post(BLOC - 1)
        ft_back(None, BLOC - 1)


def _fold_weights(inputs):
    g = np.asarray(inputs["ln_g"], np.float32)
    bb = np.asarray(inputs["ln_b"], np.float32)
    bn_g = np.asarray(inputs["bn_g"], np.float32)
    bn_b = np.asarray(inputs["bn_b"], np.float32)
    bn_mean = np.asarray(inputs["bn_mean"], np.float32)
    bn_var = np.asarray(inputs["bn_var"], np.float32)
    Wr = np.asarray(inputs["Wr"], np.float32)
    br = np.asarray(inputs["br"], np.float32)
    W_down = np.asarray(inputs["W_down"], np.float32)
    b_down = np.asarray(inputs["b_down"], np.float32)
    W_up = np.asarray(inputs["W_up"], np.float32)
    b_up = np.asarray(inputs["b_up"], np.float32)

    q = 1.0 / np.sqrt(bn_var + np.float32(EPS))
    wr_f = ((g * q * bn_g / np.float32(S))[:, None] * Wr).astype(np.float32)
    c = (((bb - bn_mean) * q * bn_g + bn_b) @ Wr + br).astype(np.float32)
    wrc = np.broadcast_to(wr_f.sum(axis=0), (P, E)).astype(np.float32)

    wd_f = (g[None, :, None] * W_down).astype(ml_dtypes.bfloat16)  # [E, D, H]
    bd_f = (b_down + np.einsum("d,edh->eh", bb, W_down)).astype(np.float32)
    wbar = wd_f.astype(np.float32).sum(axis=1)  # [E, H]
    bw = np.zeros((16, E, H), np.float32)
    bw[0] = bd_f
    bw[1] = wbar
    wu80 = np.zeros((E, 80, D), np.float32)
    wu80[:, :H] = W_up
    wu80[:, H] = b_up

    iota = np.minimum(np.arange(P) % 16, 3).astype(np.uint16).reshape(P, 1)

    return {
        "wr": np.ascontiguousarray(wr_f.reshape(DC, P, E)),
        "wrc": np.ascontiguousarray(wrc),
        "c": np.ascontiguousarray(c.reshape(1, E)),
        # mm1 pairs zT partition p (holding d = dc*P + p) with wd[p, dc*H:...]
        "wd": np.ascontiguousarray(
            wd_f.reshape(E, DC, P, H).transpose(0, 2, 1, 3).reshape(E, P, DC * H)
        ),
        "wu": np.ascontiguousarray(wu80.astype(ml_dtypes.bfloat16)),
        "bw": np.ascontiguousarray(bw.astype(ml_dtypes.bfloat16)),
        "ident": np.eye(P, dtype=ml_dtypes.bfloat16),
        "iota16": iota,
    }


def make_in_maps(inputs):
    params = _fold_weights(inputs)
    x = np.asarray(inputs["x"], np.float32)
    in_maps = []
    for i in range(NCORES):
        m = dict(params)
        m["x"] = np.ascontiguousarray(x[i * BLOC : (i + 1) * BLOC])
        in_maps.append(m)
    return in_maps


def get_nc():
    if "nc" not in _CACHE:
        _CACHE["nc"] = _build_kernel()
    return _CACHE["nc"]


def kernel(**inputs) -> np.ndarray:
    nc = get_nc()
    in_maps = make_in_maps(inputs)
    res = run_bass_kernel_spmd(nc, in_maps, core_ids=list(range(NCORES)))
    _CACHE["last_result"] = res
    out = np.concatenate(
        [np.asarray(res.results[i]["out"], np.float32) for i in range(NCORES)],
        axis=0,
    )
    return out


if __name__ == "__main__":
    nc = get_nc()
    print("build + compile OK")


# revision 22
# speedup vs baseline: 1.1402x; 1.1402x over previous
"""Trainium2 Bass kernel for nn_AdapterController (moe_routing).

Math (per sentence):
  z = LayerNorm(x) * g + b                      [S, D]
  probs = softmax(BN(mean_s z) @ Wr + br)       [E]
  idx = argmax(probs); gate = probs[idx]
  y = (relu(z @ W_down[idx] + b_down[idx]) @ W_up[idx] + b_up[idx]) * gate

Strategy: data-parallel over batch (8 sentences per core, no collectives).
DMA carries x-in (f32->bf16 SWDGE cast) + y-out; on-chip:
  - LN stats via bn_stats on bf16 x (DVE).
  - norm is FUSED into the PE transpose: zT' = xb_tile^T @ diag(rs_t)
    (regular matmuls into f32 PSUM). The mean-shift term (-mu*rs = nmr)
    is folded into mm1 as a K=2 rank-2 matmul with a host-precomputed
    [b_down ; colsum(W_down)] table, and into the router logits with a
    host colsum(Wr) constant.
  - zT PSUM->SBUF copies on ACT with accum_out => router token-sums free.
  - expert selection via GPSIMD ap_gather with an on-device argmax index
    (runtime-register DMA offsets crash under this runtime; ap_gather
    reads the index from SBUF instead). Gate is applied in the y copies.
  - mm1 -> hT psum, relu-cast on ACT; mm2 -> y psum, gate-cast copies on
    ACT/DVE; bf16->f32 cast in the output DMA (SWDGE on Pool).
  - emission is software-pipelined (front(b); back(b-1)).
"""

import sys

if "/opt/trn_rl_repo" not in sys.path:
    sys.path.insert(0, "/opt/trn_rl_repo")

from contextlib import ExitStack

import ml_dtypes
import numpy as np

import concourse.bacc as bacc
import concourse.bass as bass
import concourse.tile as tile
from concourse import mybir
from concourse.bass_utils import run_bass_kernel_spmd

B, S, D, H, E = 64, 1024, 1024, 64, 8
NCORES = 8
BLOC = B // NCORES
P = 128
TC = S // P  # token chunks per sentence
DC = D // P  # d chunks
EPS = 1e-5
FP32 = mybir.dt.float32
BF16 = mybir.dt.bfloat16
U16 = mybir.dt.uint16
I16 = mybir.dt.int16

N_Y_DVE = 6  # of 16 y copies on DVE (rest ACT)

_CACHE = {}


def _build_kernel():
    nc = bacc.Bacc(
        "TRN2",
        target_bir_lowering=False,
        debug=False,
        enable_asserts=False,
        num_devices=NCORES,
    )
    x_ext = nc.dram_tensor("x", [BLOC, S, D], FP32, kind="ExternalInput").ap()
    wr_ext = nc.dram_tensor("wr", [DC, P, E], FP32, kind="ExternalInput").ap()
    wrc_ext = nc.dram_tensor("wrc", [P, E], FP32, kind="ExternalInput").ap()
    c_ext = nc.dram_tensor("c", [1, E], FP32, kind="ExternalInput").ap()
    wd_ext = nc.dram_tensor("wd", [E, P, DC * H], BF16, kind="ExternalInput").ap()
    wu_ext = nc.dram_tensor("wu", [E, 80, D], BF16, kind="ExternalInput").ap()
    bw_ext = nc.dram_tensor("bw", [16, E, H], BF16, kind="ExternalInput").ap()
    ident_ext = nc.dram_tensor("ident", [P, P], BF16, kind="ExternalInput").ap()
    iota_ext = nc.dram_tensor("iota16", [P, 1], U16, kind="ExternalInput").ap()
    out_ext = nc.dram_tensor("out", [BLOC, S, D], FP32, kind="ExternalOutput").ap()

    with tile.TileContext(nc) as tc:
        _body(tc, out_ext, x_ext, wr_ext, wrc_ext, c_ext, wd_ext, wu_ext,
              bw_ext, ident_ext, iota_ext)

    nc.compile()
    return nc


def _body(tc, out_ext, x_ext, wr_ext, wrc_ext, c_ext, wd_ext, wu_ext,
          bw_ext, ident_ext, iota_ext):
    nc = tc.nc
    with ExitStack() as ctx:
        consts = ctx.enter_context(tc.tile_pool(name="consts", bufs=1))
        x_pool = ctx.enter_context(tc.tile_pool(name="xp", bufs=4))
        zt_pool = ctx.enter_context(tc.tile_pool(name="ztp", bufs=2))
        y_pool = ctx.enter_context(tc.tile_pool(name="yp", bufs=3))
        w_pool = ctx.enter_context(tc.tile_pool(name="wp", bufs=2))
        st_pool = ctx.enter_context(tc.tile_pool(name="stp", bufs=3))
        pp_zt = ctx.enter_context(tc.tile_pool(name="ppzt", bufs=2, space="PSUM"))
        pp_y = ctx.enter_context(tc.tile_pool(name="ppy", bufs=3, space="PSUM"))
        pp_r = ctx.enter_context(tc.tile_pool(name="ppr", bufs=1, space="PSUM"))

        # constants
        ones_row = consts.tile([1, P], FP32)
        nc.vector.memset(ones_row, 1.0)
        one_f32 = consts.tile([1, 1], FP32)
        nc.vector.memset(one_f32, 1.0)
        c_sb = consts.tile([1, E], FP32)
        nc.sync.dma_start(c_sb, c_ext)
        wr_sb = consts.tile([P, DC, E], FP32)
        nc.sync.dma_start(wr_sb, wr_ext.rearrange("dc p e -> p dc e"))
        wrc_sb = consts.tile([P, E], FP32)
        nc.sync.dma_start(wrc_sb, wrc_ext)
        ident_sb = consts.tile([P, P], BF16)
        nc.sync.dma_start(ident_sb, ident_ext)
        iota16 = consts.tile([P, 1], U16)
        nc.sync.dma_start(iota16, iota_ext)
        bw_all = consts.tile([16, E, H], BF16)
        nc.sync.dma_start(bw_all, bw_ext)
        wd_all = consts.tile([P, E, DC * H], BF16)
        nc.sync.dma_start(wd_all, wd_ext.rearrange("e p x -> p e x"))
        wu_all = consts.tile([80, E, D], BF16)
        nc.sync.dma_start(wu_all, wu_ext.rearrange("e h d -> h e d"))
        # persistent parity tiles: hT (ones row) and [ones ; nmr] K=2 rhs
        hT_tiles, onmr_tiles = [], []
        for i in range(2):
            hT_p = consts.tile([H + 1, S], BF16, tag=f"hT{i}")
            nc.vector.memset(hT_p[H : H + 1], 1.0)
            hT_tiles.append(hT_p)
            onmr_p = consts.tile([2, S], BF16, tag=f"onmr{i}")
            nc.vector.memset(onmr_p[0:1], 1.0)
            onmr_tiles.append(onmr_p)

        state0 = {}
        state = {}
        state2 = {}
        xtiles = {}

        def load(b):
            # ---- load x as bf16 via SWDGE cast (Pool) ----
            if b >= BLOC:
                return
            x_src = x_ext[b].rearrange("(tc p) d -> p tc d", p=P)
            xb = x_pool.tile([P, TC, D], BF16, tag="xb")
            for i in range(4):
                nc.gpsimd.dma_start(
                    xb[:, 2 * i : 2 * i + 2], x_src[:, 2 * i : 2 * i + 2]
                )
            xtiles[b] = xb

        def front_stats(b):
            xb = xtiles[b]

            # sentence 0 runs the chain per half so PE transposes start early
            mv = st_pool.tile([P, TC, 2], FP32)
            rs = st_pool.tile([P, TC], FP32)
            vv = st_pool.tile([P, TC], FP32)
            tnw = st_pool.tile([P, TC], FP32)
            nmr = st_pool.tile([P, TC], FP32)
            diag = st_pool.tile([P, TC, P], BF16, tag="diag")
            for ts, te in ([(0, 4), (4, 8)] if b == 0 else [(0, 8)]):
                sl = slice(ts, te)
                for t in range(ts, te):
                    stats = st_pool.tile([P, 2, 6], FP32, tag="bnstats")
                    xv = xb[:, t].rearrange("p (g f) -> p g f", g=2)
                    nc.vector.bn_stats(stats[:, 0], xv[:, 0])
                    nc.vector.bn_stats(stats[:, 1], xv[:, 1])
                    nc.vector.bn_aggr(mv[:, t], stats)
                nc.vector.tensor_scalar_add(vv[:, sl], mv[:, sl, 1], float(EPS))
                nc.vector.tensor_scalar(
                    rs[:, sl], vv[:, sl], -0.5, 1.5,
                    mybir.AluOpType.mult, mybir.AluOpType.add
                )
                for _ in range(2):
                    nc.vector.tensor_mul(tnw[:, sl], rs[:, sl], rs[:, sl])
                    nc.vector.tensor_mul(tnw[:, sl], tnw[:, sl], vv[:, sl])
                    nc.vector.tensor_scalar(
                        tnw[:, sl], tnw[:, sl], -0.5, 1.5,
                        mybir.AluOpType.mult, mybir.AluOpType.add
                    )
                    nc.vector.tensor_mul(rs[:, sl], rs[:, sl], tnw[:, sl])
                nc.vector.tensor_mul(nmr[:, sl], mv[:, sl, 0], rs[:, sl])
                nc.vector.tensor_scalar_mul(nmr[:, sl], nmr[:, sl], -1.0)
                for t in range(ts, te):
                    nc.vector.tensor_scalar_mul(
                        diag[:, t], ident_sb, rs[:, t : t + 1]
                    )

            # nmr summaries: per-partition partial sums (for router corr)
            nmr_part = st_pool.tile([P, 1], FP32)
            nc.vector.reduce_sum(nmr_part, nmr, axis=mybir.AxisListType.X)
            state0[b] = (rs, nmr, nmr_part, diag)

        def ft_back(b, bp):
            # interleaves front(b)'s fused norm-transposes with back(bp)'s
            # mm1/mm2 so the PE queue is one long ready stream
            chunks = back_chunks(bp)

            def bc(i):
                if i < len(chunks):
                    chunks[i]()

            if b is None:
                for i in range(len(chunks)):
                    bc(i)
                return
            xb = xtiles.pop(b)
            rs, nmr, nmr_part, diag = state0.pop(b)
            # ---- fused norm+transpose (PE) + ACT copies w/ accum ----
            zT_sb = zt_pool.tile([P, DC, S], BF16)
            sacc = st_pool.tile([P, DC], FP32)
            for dc in range(DC):
                pzt = pp_zt.tile([P, S], FP32, tag="zt")
                for t in range(TC):
                    nc.tensor.matmul(
                        pzt[:, t * P : (t + 1) * P],
                        xb[:, t, dc * P : (dc + 1) * P],
                        diag[:, t],
                        start=True,
                        stop=True,
                    )
                nc.scalar.activation(
                    zT_sb[:, dc],
                    pzt,
                    mybir.ActivationFunctionType.Copy,
                    accum_out=sacc[:, dc : dc + 1],
                )
                if dc % 2 == 1:
                    bc(dc // 2)

            # ---- logits = s @ Wr_f + c + nmrsum*colsum(Wr_f)  (PE) ----
            ps_l = pp_r.tile([1, E], FP32, tag="ps_small")
            for dc in range(DC):
                nc.tensor.matmul(
                    ps_l, sacc[:, dc : dc + 1], wr_sb[:, dc], start=(dc == 0),
                    stop=False,
                )
            nc.tensor.matmul(ps_l, nmr_part, wrc_sb, start=False, stop=False)
            nc.tensor.matmul(ps_l, one_f32, c_sb, start=False, stop=True)
            logits_sb = st_pool.tile([1, E], FP32)
            nc.scalar.copy(logits_sb, ps_l)
            for i in range(4, len(chunks)):
                bc(i)

            state[b] = (zT_sb, logits_sb, nmr)

        def front_post(b):
            zT_sb, logits_sb, nmr = state.pop(b)
            # nmr as a [1, S] bf16 row (token-major flatten) for the mm1 K=2 bias
            nmr_bf = st_pool.tile([P, TC], BF16, tag="nmrbf")
            nc.vector.tensor_copy(nmr_bf, nmr)
            ps_nt = pp_r.tile([TC, P], BF16, tag="ps_small")
            nc.tensor.transpose(ps_nt, nmr_bf, ident_sb)
            nmrT = st_pool.tile([TC, P], BF16, tag="nmrT")
            nc.scalar.copy(nmrT, ps_nt)
            onmr = onmr_tiles[b % 2]
            nc.sync.dma_start(
                onmr[1:2].rearrange("a (tc p) -> a tc p", p=P), nmrT
            )
            # ---- gating + expert index (DVE) ----
            mx8 = st_pool.tile([1, E], FP32)
            ix8 = st_pool.tile([1, E], U16)
            nc.vector.max_with_indices(mx8, ix8, logits_sb)
            u = st_pool.tile([1, E], FP32)
            nc.vector.tensor_scalar(
                u, logits_sb, mx8[0:1, 0:1], None, mybir.AluOpType.subtract
            )
            ex = st_pool.tile([1, E], FP32)
            nc.vector.tensor_scalar(
                ex, u, 0.25, 1.0, mybir.AluOpType.mult, mybir.AluOpType.add
            )
            for coef in (3.0, 2.0, 1.0):
                nc.vector.tensor_mul(ex, ex, u)
                nc.vector.tensor_scalar(
                    ex, ex, 1.0 / coef, 1.0, mybir.AluOpType.mult, mybir.AluOpType.add
                )
            denom = st_pool.tile([1, 1], FP32)
            nc.vector.tensor_reduce(
                denom, ex, axis=mybir.AxisListType.X, op=mybir.AluOpType.add
            )
            gate = st_pool.tile([1, 1], FP32)
            nc.vector.reciprocal(gate, denom)
            ix4 = st_pool.tile([1, 1], U16)
            nc.vector.tensor_scalar_mul(ix4, ix8[0:1, 0:1], 4)

            # index row: 4*idx + (p%16 capped at 3), broadcast to 128 parts
            ixb = st_pool.tile([P, 1], U16)
            nc.gpsimd.partition_broadcast(ixb, ix4)
            nc.vector.tensor_add(ixb, ixb, iota16)

            # ---- expert selection via ap_gather (Pool) ----
            wd_eff = w_pool.tile([P, 4, P], BF16, tag="wd")
            nc.gpsimd.ap_gather(
                wd_eff,
                wd_all.rearrange("p e (q f) -> p (e q) f", q=4),
                ixb.bitcast(I16),
                channels=P, num_elems=4 * E, d=P, num_idxs=4,
            )
            wu_eff = w_pool.tile([80, 4, 256], BF16, tag="wu")
            nc.gpsimd.ap_gather(
                wu_eff,
                wu_all.rearrange("h e (q f) -> h (e q) f", q=4),
                ixb[0:80].bitcast(I16),
                channels=80, num_elems=4 * E, d=256, num_idxs=4,
            )
            bw_sel = w_pool.tile([16, 4, 16], BF16, tag="bw")
            nc.gpsimd.ap_gather(
                bw_sel,
                bw_all.rearrange("c e (q f) -> c (e q) f", q=4),
                ixb[0:16].bitcast(I16),
                channels=16, num_elems=4 * E, d=16, num_idxs=4,
            )

            # gate broadcast to 128 partitions (Pool, tiny)
            gate_bc = st_pool.tile([P, 1], FP32)
            nc.gpsimd.partition_broadcast(gate_bc, gate)

            state2[b] = (zT_sb, wd_eff, wu_eff, bw_sel, gate_bc)

        def back_chunks(b):
            if b is None:
                return []
            zT_sb, wd_eff, wu_eff, bw_sel, gate_bc = state2.pop(b)
            hT = hT_tiles[b % 2]
            onmr = onmr_tiles[b % 2]
            wd_flat = wd_eff.rearrange("p q f -> p (q f)")
            wu_flat = wu_eff.rearrange("p q f -> p (q f)")
            bw_flat = bw_sel.rearrange("p q f -> p (q f)")
            y_dst = out_ext[b].rearrange("(tc p) d -> p tc d", p=P)
            y_hs = []
            for i in range(2):
                y_half = y_pool.tile([P, TC // 2, D], BF16, tag="yh")
                y_hs.append(y_half)

            def mm1(half):
                hsl = slice(half * 512, (half + 1) * 512)
                ps_hT = pp_y.tile([H, 512], FP32, tag="ps")
                for dc in range(DC):
                    nc.tensor.matmul(
                        ps_hT,
                        wd_flat[:, dc * H : (dc + 1) * H],
                        zT_sb[:, dc, hsl],
                        start=(dc == 0),
                        stop=False,
                    )
                nc.tensor.matmul(
                    ps_hT, bw_flat[0:2], onmr[:, hsl], start=False, stop=True
                )
                nc.scalar.activation(
                    hT[0:H, hsl], ps_hT, mybir.ActivationFunctionType.Relu
                )

            def mm2(trange, store_half=None):
                for t in trange:
                    y_sb = y_hs[t // (TC // 2)]
                    yt = t % (TC // 2)
                    for half in range(2):
                        hs = slice(half * 512, (half + 1) * 512)
                        ps_y = pp_y.tile([P, 512], FP32, tag="ps")
                        nc.tensor.matmul(
                            ps_y,
                            hT[:, t * P : (t + 1) * P],
                            wu_flat[0 : H + 1, hs],
                            start=True,
                            stop=True,
                        )
                        if (2 * t + half) % 16 < N_Y_DVE:
                            nc.vector.tensor_scalar_mul(
                                y_sb[:, yt, hs], ps_y, gate_bc
                            )
                        else:
                            nc.scalar.mul(y_sb[:, yt, hs], ps_y, gate_bc)
                    if b == BLOC - 1 and t % 2 == 1:
                        q = t // 2
                        nc.gpsimd.dma_start(
                            y_dst[:, 2 * q : 2 * q + 2],
                            y_hs[q // 2][:, 2 * (q % 2) : 2 * (q % 2) + 2],
                        )
                if b != BLOC - 1 and store_half is not None:
                    h0 = store_half * (TC // 2)
                    nc.gpsimd.dma_start(
                        y_dst[:, h0 : h0 + TC // 2], y_hs[store_half]
                    )

            return [
                lambda: mm1(0),
                lambda: mm1(1),
                lambda: mm2(range(0, 4), store_half=0),
                lambda: mm2(range(4, 8), store_half=1),
            ]

        load(0)
        load(1)
        front_stats(0)
        load(2)
        for k in range(BLOC):
            if k >= 1:
                front_post(k - 1)
            if k + 1 < BLOC:
                front_stats(k + 1)
            ft_back(k, k - 1 if k >= 1 else None)
            load(k + 3)
        front_post(BLOC - 1)
        ft_back(None, BLOC - 1)


def _fold_weights(inputs):
    g = np.asarray(inputs["ln_g"], np.float32)
    bb = np.asarray(inputs["ln_b"], np.float32)
    bn_g = np.asarray(inputs["bn_g"], np.float32)
    bn_b = np.asarray(inputs["bn_b"], np.float32)
    bn_mean = np.asarray(inputs["bn_mean"], np.float32)
    bn_var = np.asarray(inputs["bn_var"], np.float32)
    Wr = np.asarray(inputs["Wr"], np.float32)
    br = np.asarray(inputs["br"], np.float32)
    W_down = np.asarray(inputs["W_down"], np.float32)
    b_down = np.asarray(inputs["b_down"], np.float32)
    W_up = np.asarray(inputs["W_up"], np.float32)
    b_up = np.asarray(inputs["b_up"], np.float32)

    q = 1.0 / np.sqrt(bn_var + np.float32(EPS))
    wr_f = ((g * q * bn_g / np.float32(S))[:, None] * Wr).astype(np.float32)
    c = (((bb - bn_mean) * q * bn_g + bn_b) @ Wr + br).astype(np.float32)
    wrc = np.broadcast_to(wr_f.sum(axis=0), (P, E)).astype(np.float32)

    wd_f = (g[None, :, None] * W_down).astype(ml_dtypes.bfloat16)  # [E, D, H]
    bd_f = (b_down + np.einsum("d,edh->eh", bb, W_down)).astype(np.float32)
    wbar = wd_f.astype(np.float32).sum(axis=1)  # [E, H]
    bw = np.zeros((16, E, H), np.float32)
    bw[0] = bd_f
    bw[1] = wbar
    wu80 = np.zeros((E, 80, D), np.float32)
    wu80[:, :H] = W_up
    wu80[:, H] = b_up

    iota = np.minimum(np.arange(P) % 16, 3).astype(np.uint16).reshape(P, 1)

    return {
        "wr": np.ascontiguousarray(wr_f.reshape(DC, P, E)),
        "wrc": np.ascontiguousarray(wrc),
        "c": np.ascontiguousarray(c.reshape(1, E)),
        # mm1 pairs zT partition p (holding d = dc*P + p) with wd[p, dc*H:...]
        "wd": np.ascontiguousarray(
            wd_f.reshape(E, DC, P, H).transpose(0, 2, 1, 3).reshape(E, P, DC * H)
        ),
        "wu": np.ascontiguousarray(wu80.astype(ml_dtypes.bfloat16)),
        "bw": np.ascontiguousarray(bw.astype(ml_dtypes.bfloat16)),
        "ident": np.eye(P, dtype=ml_dtypes.bfloat16),
        "iota16": iota,
    }


def make_in_maps(inputs):
    params = _fold_weights(inputs)
    x = np.asarray(inputs["x"], np.float32)
    in_maps = []
    for i in range(NCORES):
        m = dict(params)
        m["x"] = np.ascontiguousarray(x[i * BLOC : (i + 1) * BLOC])
        in_maps.append(m)
    return in_maps


def get_nc():
    if "nc" not in _CACHE:
        _CACHE["nc"] = _build_kernel()
    return _CACHE["nc"]


def kernel(**inputs) -> np.ndarray:
    nc = get_nc()
    in_maps = make_in_maps(inputs)
    res = run_bass_kernel_spmd(nc, in_maps, core_ids=list(range(NCORES)))
    _CACHE["last_result"] = res
    out = np.concatenate(
        [np.asarray(res.results[i]["out"], np.float32) for i in range(NCORES)],
        axis=0,
    )
    return out


if __name__ == "__main__":
    nc = get_nc()
    print("build + compile OK")
